# revision 29
# baseline (speedup 1.0000x reference)
"""Tensor-parallel MHA prefill kernel for 8 TRN2 NeuronCores.

Sharding: heads across cores (4 Q heads + 1 KV head per core).
Per core: QKV projection (bf16 matmuls, fp32 accum), interleaved RoPE,
causal attention in scores-transposed orientation (softmax denominators
via an appended ones-column in the AV matmul, accumulated over 4-j-tile
subgroups into SBUF), pair-split AllToAll overlapped with the split
output projection for this core's 256 sequence rows. Host only
slices/transposes/casts weights and the input, and concatenates the 8
output row-blocks.
"""
import os
import numpy as np
import ml_dtypes

N_CORES = 8
S = 2048
D = 2048
NH = 32
HD = 64
HPC = NH // N_CORES      # 4 q heads per core
QW = HPC * HD            # 256
QK = QW + HD             # 320
QKV = QW + 2 * HD        # 384
SCALE = 1.0 / np.sqrt(HD)

ST = 128
NS = S // ST             # 16
DT = 128
ND = D // DT             # 16
NXB = 8                  # stripes transposed via xbar DMA (rest via PE)
IC = 512
NCH = S // IC            # 4
SROWS = S // N_CORES     # 256
HD1 = HD + 1             # 65

_CACHE = {}


def _build():
    from concourse import bacc
    import concourse.mybir as mybir
    from concourse.tile import TileContext
    from concourse.masks import make_identity

    dt = mybir.dt
    Exp = mybir.ActivationFunctionType.Exp
    nc = bacc.Bacc("TRN2", target_bir_lowering=False, debug=False,
                   num_devices=N_CORES)

    xstr = nc.declare_dram_parameter("xstr", [ND, S, DT], dt.bfloat16,
                                     isOutput=False)
    wqkvT = nc.declare_dram_parameter("wqkvT", [128, ND * QKV], dt.bfloat16,
                                      isOutput=False)
    woT = nc.declare_dram_parameter("woT", [128, ND * D], dt.bfloat16,
                                    isOutput=False)
    cos2 = nc.declare_dram_parameter("cos2", [128, NS * QK], dt.float32,
                                     isOutput=False)
    sin2 = nc.declare_dram_parameter("sin2", [128, NS * QK], dt.float32,
                                     isOutput=False)
    out = nc.declare_dram_parameter("out", [SROWS, D], dt.float32, isOutput=True)

    a2a_in = [nc.dram_tensor(f"a2a_in{p}", [N_CORES, 128, SROWS], dt.bfloat16)
              for p in range(2)]
    a2a_out = [nc.dram_tensor(f"a2a_out{p}", [N_CORES, 128, SROWS], dt.bfloat16)
               for p in range(2)]

    with TileContext(nc) as tc:
        const = tc.alloc_tile_pool(name="const", bufs=1)
        ident = const.tile([128, 128], dt.bfloat16, tag="ident")
        make_identity(nc, ident)
        dmask = const.tile([128, 128], dt.bfloat16, tag="dmask")
        nc.gpsimd.memset(dmask[:], 1.0)
        nc.gpsimd.affine_select(
            out=dmask[:], in_=dmask[:], compare_op=mybir.AluOpType.is_ge,
            fill=0.0, base=0, pattern=[[1, 128]], channel_multiplier=-1)

        pers = tc.alloc_tile_pool(name="pers", bufs=1)
        wq_sb = pers.tile([128, ND * QKV], dt.bfloat16, tag="wq")
        cs_sb = pers.tile([128, NS * QK], dt.float32, tag="cs")
        sn_sb = pers.tile([128, NS * QK], dt.float32, tag="sn")
        qT2 = [pers.tile([128, S], dt.bfloat16, name=f"qT{p}", tag=f"qT{p}")
               for p in range(2)]
        kT2 = pers.tile([128, S], dt.bfloat16, tag="kT2")
        v_aug = pers.tile([128, NS * HD1], dt.bfloat16, tag="vaug")
        nc.gpsimd.memset(v_aug[:], 1.0)

        # ---- phase 1+2: hybrid x transposes (8 xbar on sync, 8 PE) + QKV ----
        with (
            tc.tile_pool(name="xt", bufs=1) as xt_pool,
            tc.tile_pool(name="xn", bufs=4) as xn_pool,
        ):
            xT = [xt_pool.tile([128, S], dt.bfloat16, name=f"xT{i}",
                               tag=f"xT{i}") for i in range(ND)]
            nc.gpsimd.dma_start(out=wq_sb[:], in_=wqkvT[:])
            for i in range(NXB):
                nc.sync.dma_start(out=xT[i][:], in_=xstr[i], transpose=True)
            nc.gpsimd.dma_start(out=cs_sb[:], in_=cos2[:])
            nc.gpsimd.dma_start(out=sn_sb[:], in_=sin2[:])

            with (
                tc.tile_pool(name="qkv_ps", bufs=5, space="PSUM") as qkv_ps,
                tc.tile_pool(name="tr_ps", bufs=3, space="PSUM") as tr_ps,
                tc.tile_pool(name="rope", bufs=3) as rope_pool,
                tc.tile_pool(name="qkrot", bufs=1) as qkrot_pool,
            ):
                for s in range(NS):
                    xn = xn_pool.tile([128, (ND - NXB) * DT], dt.bfloat16,
                                      tag="xn", name="xn")
                    nc.gpsimd.dma_start(
                        out=xn[:],
                        in_=xstr[NXB:ND, s * ST:(s + 1) * ST, :]
                        .rearrange("i s d -> s i d"))
                    for i in range(NXB, ND):
                        pt = tr_ps.tile([128, 128], dt.bfloat16, tag="tr",
                                        name="pt")
                        nc.tensor.transpose(
                            pt[:], xn[:, (i - NXB) * DT:(i - NXB + 1) * DT],
                            ident[:])
                        nc.vector.tensor_copy(xT[i][:, s * ST:(s + 1) * ST],
                                              pt[:])
                qkrot = [qkrot_pool.tile([128, QK], dt.bfloat16,
                                         name=f"qk{s}", tag=f"qk{s}")
                         for s in range(NS)]
                for s in range(NS):
                    ps = qkv_ps.tile([128, QKV], dt.float32, tag="qkv",
                                     name="qkv")
                    for i in range(ND):
                        nc.tensor.matmul(ps[:], xT[i][:, s * ST:(s + 1) * ST],
                                         wq_sb[:, i * QKV:(i + 1) * QKV],
                                         start=(i == 0), stop=(i == ND - 1))
                    qk_w = rope_pool.tile([128, QK], dt.float32, tag="qkw",
                                          name="qkw")
                    nc.scalar.copy(qk_w[:, 0:QK:2], ps[:, 1:QK:2])
                    nc.scalar.copy(qk_w[:, 1:QK:2], ps[:, 0:QK:2])
                    nc.vector.tensor_copy(v_aug[:, s * HD1:s * HD1 + HD],
                                          ps[:, QK:QKV])
                    t1 = rope_pool.tile([128, QK], dt.float32, tag="t1",
                                        name="t1")
                    t2 = rope_pool.tile([128, QK], dt.float32, tag="t2",
                                        name="t2")
                    nc.vector.tensor_mul(t1[:], ps[:, 0:QK],
                                         cs_sb[:, s * QK:(s + 1) * QK])
                    nc.vector.tensor_mul(t2[:], qk_w[:],
                                         sn_sb[:, s * QK:(s + 1) * QK])
                    nc.vector.tensor_add(qkrot[s][:], t1[:], t2[:])
                    for p in range(2):
                        pt = tr_ps.tile([128, 128], dt.bfloat16, tag="tr",
                                        name="pt")
                        nc.tensor.transpose(
                            pt[:], qkrot[s][:, p * 128:(p + 1) * 128],
                            ident[:])
                        nc.vector.tensor_copy(qT2[p][:, s * ST:(s + 1) * ST],
                                              pt[:])
                    pt = tr_ps.tile([128, 128], dt.bfloat16, tag="tr",
                                    name="pt")
                    nc.tensor.transpose(pt[0:HD, :], qkrot[s][:, QW:QK],
                                        ident[:])
                    nc.vector.tensor_copy(kT2[0:HD, s * ST:(s + 1) * ST],
                                          pt[0:HD, :])
                    nc.vector.tensor_copy(kT2[HD:128, s * ST:(s + 1) * ST],
                                          pt[0:HD, :])

        # ---- preload woT while attention runs ----
        wo_pool = tc.alloc_tile_pool(name="wo_sb", bufs=1)
        wo_sb = wo_pool.tile([128, ND * D], dt.bfloat16, tag="wo")
        nc.gpsimd.dma_start(out=wo_sb[:], in_=woT[:])

        # ---- phase 3: attention (pair-major; merged hh scores+exp;
        #      AV in 4-jt subgroups into SBUF) ----
        with (
            tc.tile_pool(name="sc_ps", bufs=2, space="PSUM") as sc_ps,
            tc.tile_pool(name="av_ps", bufs=1, space="PSUM") as av_ps,
            tc.tile_pool(name="yt_ps", bufs=1, space="PSUM") as yt_ps,
            tc.tile_pool(name="expT", bufs=4) as exp_pool,
            tc.tile_pool(name="acc", bufs=1) as acc_pool,
            tc.tile_pool(name="ytmp", bufs=3) as ytmp_pool,
            tc.tile_pool(name="ystage", bufs=3) as ystage_pool,
        ):
            for p in range(2):
                accs = {}
                for hh in range(2):
                    accs[hh] = acc_pool.tile([128, 4 * HD1], dt.float32,
                                             name=f"acc{hh}", tag=f"acc{hh}")
                for c in range(NCH):
                    ngrp = c + 1
                    ets = {}
                    for g in range(ngrp):
                        for jt in range(4 * g, 4 * g + 4):
                            toff = jt - 4 * c
                            lo = max(toff, 0) * 128
                            w = IC - lo
                            ps_s = sc_ps.tile([128, 2 * IC], dt.float32,
                                              tag="sc", name="sc")
                            for hh in range(2):
                                nc.tensor.matmul(
                                    ps_s[:, hh * IC:hh * IC + w],
                                    kT2[hh * HD:hh * HD + HD,
                                        jt * ST:(jt + 1) * ST],
                                    qT2[p][hh * HD:hh * HD + HD,
                                           c * IC + lo:(c + 1) * IC],
                                    start=True, stop=True,
                                    tile_position=(hh * HD, 0))
                            et = exp_pool.tile([128, 2 * IC], dt.bfloat16,
                                               name=f"et{jt % 6}",
                                               tag=f"et{jt % 6}")
                            ets[jt] = et
                            src_ap = ps_s[:, 0:2 * IC].rearrange(
                                "p (h w) -> p h w", h=2)[:, :, 0:w]
                            dst_ap = et[:].rearrange(
                                "p (h w) -> p h w", h=2)[:, :, lo:IC]
                            nc.scalar.activation(dst_ap, src_ap, Exp,
                                                 scale=float(SCALE))
                            if toff >= 0:
                                for hh in range(2):
                                    nc.vector.tensor_mul(
                                        et[:, hh * IC + lo:hh * IC + lo + 128],
                                        et[:, hh * IC + lo:hh * IC + lo + 128],
                                        dmask[:])
                        for hh in range(2):
                            pavt = av_ps.tile([128, 4 * HD1], dt.float32,
                                              name=f"av{hh}", tag=f"av{hh}")
                            for t in range(4):
                                j0 = 4 * g
                                j1 = min(4 * g + 3, 4 * c + t)
                                for jt in range(j0, j1 + 1):
                                    nc.tensor.matmul(
                                        pavt[:, t * HD1:t * HD1 + HD1],
                                        ets[jt][:, hh * IC + t * 128:
                                                hh * IC + (t + 1) * 128],
                                        v_aug[:, jt * HD1:(jt + 1) * HD1],
                                        start=(jt == j0), stop=(jt == j1))
                            acc = accs[hh]
                            if g == 0:
                                nc.vector.tensor_copy(acc[:], pavt[:])
                            else:
                                nc.vector.tensor_add(acc[:], acc[:], pavt[:])
                    ys4 = ystage_pool.tile([128, IC], dt.bfloat16, tag="ys",
                                           name="ys")
                    for t in range(4):
                        ypair = ytmp_pool.tile([128, 128], dt.bfloat16,
                                               tag="yp", name="yp")
                        for hh in range(2):
                            base = t * HD1
                            rec = ytmp_pool.tile([128, 1], dt.float32,
                                                 tag="rec", name="rec")
                            nc.vector.reciprocal(
                                rec[:], accs[hh][:, base + HD:base + HD + 1])
                            nc.vector.tensor_scalar_mul(
                                ypair[:, hh * HD:(hh + 1) * HD],
                                accs[hh][:, base:base + HD], rec[:])
                        pt = yt_ps.tile([128, 128], dt.bfloat16, tag="yt",
                                        name="pt")
                        nc.tensor.transpose(pt[:], ypair[:], ident[:])
                        nc.vector.tensor_copy(ys4[:, t * 128:(t + 1) * 128],
                                              pt[:])
                    eng = nc.sync if c % 2 == 0 else nc.scalar
                    for j in range(2):
                        eng.dma_start(
                            out=a2a_in[p][2 * c + j, :, :],
                            in_=ys4[:, j * SROWS:(j + 1) * SROWS])
                nc.gpsimd.collective_compute(
                    "AllToAll", mybir.AluOpType.bypass,
                    replica_groups=[list(range(N_CORES))],
                    ins=[a2a_in[p][:]], outs=[a2a_out[p][:]])

        # ---- phase 4: output projection (even m-tiles overlap A2A#2) ----
        with (
            tc.tile_pool(name="ytf", bufs=1) as ytf_pool,
            tc.tile_pool(name="o_ps", bufs=1, space="PSUM") as o_ps,
            tc.tile_pool(name="o_sb", bufs=3) as o_sb,
        ):
            ytf = {}
            ps_os = {}
            for p in range(2):
                flat = a2a_out[p][:].rearrange("r m s -> (r m) s")
                for r in range(N_CORES):
                    mt = 2 * r + p
                    ytf[mt] = ytf_pool.tile([128, SROWS], dt.bfloat16,
                                            name=f"ytf{mt}", tag=f"ytf{mt}")
                    nc.sync.dma_start(out=ytf[mt][:],
                                      in_=flat[r * 128:(r + 1) * 128, :])
                for st in range(2):
                    for nch in range(4):
                        if p == 0:
                            ps_os[(st, nch)] = o_ps.tile(
                                [128, 512], dt.float32,
                                name=f"o{st}{nch}", tag=f"o{st}{nch}")
                        ps_o = ps_os[(st, nch)]
                        for r in range(N_CORES):
                            mt = 2 * r + p
                            nc.tensor.matmul(
                                ps_o[:], ytf[mt][:, st * 128:(st + 1) * 128],
                                wo_sb[:, mt * D + nch * 512:
                                      mt * D + (nch + 1) * 512],
                                start=(p == 0 and r == 0),
                                stop=(p == 1 and r == N_CORES - 1))
                        if p == 1:
                            ob = o_sb.tile([128, 512], dt.float32, tag="ob",
                                           name="ob")
                            nc.scalar.copy(ob[:], ps_o[:])
                            nc.scalar.dma_start(
                                out=out[st * 128:(st + 1) * 128,
                                        nch * 512:(nch + 1) * 512],
                                in_=ob[:])

        wo_pool.release()
        pers.release()
        const.release()

    nc.compile()
    return nc


def _numpy_reference(x, freqs_cos, freqs_sin, input_pos, wq, wk, wv, wo,
                     k_cache, v_cache):
    B, S_, _ = x.shape
    NKV = 8
    n_rep = NH // NKV

    def rope(t, cos, sin):
        tr = t[..., 0::2]
        ti = t[..., 1::2]
        c = cos[None, :, None, :]
        s = sin[None, :, None, :]
        o = np.stack([tr * c - ti * s, tr * s + ti * c], axis=-1)
        return o.reshape(t.shape)

    q = (x @ wq.T).reshape(B, S_, NH, HD)
    k = (x @ wk.T).reshape(B, S_, NKV, HD)
    v = (x @ wv.T).reshape(B, S_, NKV, HD)
    q = rope(q, freqs_cos, freqs_sin).transpose(0, 2, 1, 3)
    k = rope(k, freqs_cos, freqs_sin).transpose(0, 2, 1, 3)
    v = v.transpose(0, 2, 1, 3)
    k_full = np.array(k_cache)
    v_full = np.array(v_cache)
    k_full[:, :, input_pos] = k
    v_full[:, :, input_pos] = v
    mask = np.tril(np.ones((k_full.shape[2], k_full.shape[2]), bool))[input_pos]
    k_rep = np.repeat(k_full, n_rep, axis=1)
    v_rep = np.repeat(v_full, n_rep, axis=1)
    sc = np.einsum("bhsd,bhtd->bhst", q, k_rep) * SCALE
    sc = np.where(mask[None, None], sc, -np.inf)
    sc = sc - sc.max(axis=-1, keepdims=True)
    e = np.exp(sc)
    attn = e / e.sum(axis=-1, keepdims=True)
    y = np.einsum("bhst,bhtd->bhsd", attn, v_rep)
    y = y.transpose(0, 2, 1, 3).reshape(B, S_, NH * HD)
    return (y @ wo.T).astype(np.float32)


def _pair_expand(a, sign_odd=False):
    S_, n = a.shape
    o = np.empty((S_, 2 * n), np.float32)
    o[:, 0::2] = -a if sign_odd else a
    o[:, 1::2] = a
    return o


def _fold_stiles(a):
    W = a.shape[1]
    return np.ascontiguousarray(
        a.reshape(NS, 128, W).transpose(1, 0, 2).reshape(128, NS * W))


def kernel(x, freqs_cos, freqs_sin, input_pos, wq, wk, wv, wo,
           k_cache, v_cache):
    ipos = np.asarray(input_pos)
    if not np.array_equal(ipos, np.arange(S, dtype=ipos.dtype)):
        return _numpy_reference(np.asarray(x, np.float32),
                                np.asarray(freqs_cos), np.asarray(freqs_sin),
                                ipos, np.asarray(wq), np.asarray(wk),
                                np.asarray(wv), np.asarray(wo),
                                np.asarray(k_cache), np.asarray(v_cache))

    from concourse.bass_utils import run_bass_kernel_spmd

    if "nc" not in _CACHE:
        _CACHE["nc"] = _build()
    nc = _CACHE["nc"]

    bf16 = ml_dtypes.bfloat16
    x2 = np.asarray(x, np.float32)[0].astype(bf16)
    xstr = np.ascontiguousarray(x2.reshape(S, ND, DT).transpose(1, 0, 2))
    cos = np.asarray(freqs_cos, np.float32)
    sin = np.asarray(freqs_sin, np.float32)
    cos2 = _fold_stiles(np.tile(_pair_expand(cos), (1, 5)))
    sin2 = _fold_stiles(np.tile(_pair_expand(sin, sign_odd=True), (1, 5)))
    woTf = np.asarray(wo, np.float32).T.astype(bf16)
    woT = np.ascontiguousarray(
        woTf.reshape(ND, 128, D).transpose(1, 0, 2).reshape(128, ND * D))

    in_maps = []
    for c in range(N_CORES):
        wq_c = np.asarray(wq, np.float32)[c * QW:(c + 1) * QW].T
        wk_c = np.asarray(wk, np.float32)[c * HD:(c + 1) * HD].T
        wv_c = np.asarray(wv, np.float32)[c * HD:(c + 1) * HD].T
        wqkvT_f = np.concatenate([wq_c, wk_c, wv_c], axis=1).astype(bf16)
        wqkvT = np.ascontiguousarray(
            wqkvT_f.reshape(ND, 128, QKV).transpose(1, 0, 2)
            .reshape(128, ND * QKV))
        in_maps.append({
            "xstr": xstr, "wqkvT": wqkvT, "woT": woT,
            "cos2": cos2, "sin2": sin2,
        })

    res = run_bass_kernel_spmd(nc, in_maps, core_ids=list(range(N_CORES)),
                               trace=bool(os.environ.get("KERNEL_TRACE")))
    _CACHE["last_res"] = res
    rows = [res.results[c]["out"] for c in range(N_CORES)]
    return np.concatenate(rows, axis=0)[None].astype(np.float32)
